# revision 61
# baseline (speedup 1.0000x reference)
"""HaciCognitiveNet Trainium2 kernel (bf16 edition).

Data-parallel over batch: B=8 -> one batch element per NeuronCore.
Activations live TRANSPOSED on-chip ([D, S], D on partitions).

Matmul inputs are bf16 (fp32 PSUM accumulation): fp32 moving operands
stream at 2 cycles/column on the PE, bf16 at 1 -- halving matmul time.
The fp32 residual stream is kept in SBUF; bf16 copies (htb) feed the PE.

LayerNorm over D (partition dim) via ones-column matmuls + Newton rsqrt
on DVE. Mean correction folded into projections as rank-1 K=1 matmuls;
negmu is computed from the sums matmuls alone so projections launch
before the rsqrt chain finishes.

The decay mask 0.99^(q-k) is separable; the per-query factor
rstd(q)*0.99^q/sqrt(dh) is DROPPED entirely (the inner LayerNorm is
invariant to a positive per-token scale of ret). The per-key scale
ks(k) = rstd(k)^2 * 0.99^-k is applied on the V drain as a per-partition
ACT scale (V tiles carry keys on partitions); ks reaches column layout
via a tiny SBUF->SBUF transpose DMA. Q and K drains are plain copies.

Engine budget per retention layer (approx): PE 33us, DVE ~24us (at-mask
drains dominate), ACT ~16us. Layer-boundary serial chains are kept under
the ~3.4us HAM idle window so the PE clock stays at 2.4GHz.
"""

import numpy as np

B, S, DIN, D, H, FF = 8, 512, 384, 512, 8, 2048
DH = D // H
N_WM, N_CORE = 2, 4
NL = N_WM + N_CORE
DECAY = 0.99
EPS = 1e-5
MAGIC = 0x5F3759DF
PT = D // 128   # 4 partition tiles of the model dim
CT = S // 128   # 4 tiles of the sequence dim

_CACHE = {}


def _lhsT_layout(w):
    """[K, M] weight -> SBUF lhsT tile layout [128, (K//128)*M]."""
    k, m = w.shape
    c = k // 128
    return np.ascontiguousarray(
        w.reshape(c, 128, m).transpose(1, 0, 2).reshape(128, c * m)
    )


def _build_program():
    import concourse.bass as bass
    import concourse.tile as tile
    from concourse import mybir, bacc
    from contextlib import ExitStack

    f32 = mybir.dt.float32
    f32r = mybir.dt.float32r
    bf16 = mybir.dt.bfloat16
    i32 = mybir.dt.int32
    AF = mybir.ActivationFunctionType
    ALU = mybir.AluOpType

    nc = bacc.Bacc("TRN2", target_bir_lowering=False, debug=False)

    XT = nc.dram_tensor("xt", [128, 3, 512], bf16, kind="ExternalInput").ap()
    INW = nc.dram_tensor("inw", [128, 3 * 512], bf16, kind="ExternalInput").ap()
    INB = nc.dram_tensor("inb", [128, 4], f32, kind="ExternalInput").ap()
    WST = nc.dram_tensor("wst", [NL, 5, 128, 2048], bf16, kind="ExternalInput").ap()
    UST = nc.dram_tensor("ust", [NL, 1, 4 * 512], bf16, kind="ExternalInput").ap()
    BG = nc.dram_tensor("bg", [NL, 128, 4], f32, kind="ExternalInput").ap()
    BO = nc.dram_tensor("bo", [NL, 128, 4], f32, kind="ExternalInput").ap()
    W1T = nc.dram_tensor("w1t", [128, 4, 2048], bf16, kind="ExternalInput").ap()
    W2T = nc.dram_tensor("w2t", [128, 16 * 512], bf16, kind="ExternalInput").ap()
    B1C = nc.dram_tensor("b1c", [128, 16], f32, kind="ExternalInput").ap()
    B2C = nc.dram_tensor("b2c", [128, 4], f32, kind="ExternalInput").ap()
    ONWC = nc.dram_tensor("onwc", [128, 4], f32, kind="ExternalInput").ap()
    ONBC = nc.dram_tensor("onbc", [128, 4], f32, kind="ExternalInput").ap()
    DKC = nc.dram_tensor("dkc", [128, 4], f32, kind="ExternalInput").ap()
    MSK = nc.dram_tensor("msk", [4, 128, 512], bf16, kind="ExternalInput").ap()
    ONESC = nc.dram_tensor("onesc", [128, 1], bf16, kind="ExternalInput").ap()
    ONESCR = nc.dram_tensor("onescr", [128, 1], f32r, kind="ExternalInput").ap()
    EPSR = nc.dram_tensor("epsr", [1, 512], f32r, kind="ExternalInput").ap()
    ONESR = nc.dram_tensor("onesr", [1, 128], f32r, kind="ExternalInput").ap()
    HOUT = nc.dram_tensor("hout", [4, 128, 512], bf16, kind="ExternalOutput").ap()

    with tile.TileContext(nc) as tc:
        with ExitStack() as ctx:
            consts = ctx.enter_context(tc.tile_pool(name="consts", bufs=1))
            wpool = ctx.enter_context(tc.tile_pool(name="wpool", bufs=10))
            wsmall = ctx.enter_context(tc.tile_pool(name="wsmall", bufs=2))
            hpool = ctx.enter_context(tc.tile_pool(name="hpool", bufs=2))
            hbpool = ctx.enter_context(tc.tile_pool(name="hbpool", bufs=2))
            apool = ctx.enter_context(tc.tile_pool(name="apool", bufs=1))
            atpool = ctx.enter_context(tc.tile_pool(name="atpool", bufs=8))
            spool = ctx.enter_context(tc.tile_pool(name="spool", bufs=6))
            sqpool = ctx.enter_context(tc.tile_pool(name="sqpool", bufs=4))
            grpool = ctx.enter_context(tc.tile_pool(name="grpool", bufs=4))
            retpool = ctx.enter_context(tc.tile_pool(name="retpool", bufs=1))
            plpool = ctx.enter_context(tc.tile_pool(name="plpool", bufs=4))
            odpool = ctx.enter_context(tc.tile_pool(name="odpool", bufs=2))
            f1pool = ctx.enter_context(tc.tile_pool(name="f1pool", bufs=3))
            psum = ctx.enter_context(tc.tile_pool(name="psum", bufs=5, space="PSUM"))
            bcps = ctx.enter_context(tc.tile_pool(name="bcps", bufs=2, space="PSUM"))
            rpsum = ctx.enter_context(tc.tile_pool(name="rpsum", bufs=1, space="PSUM"))

            # ---- input DMAs first so the PE can start ASAP ----
            inctx = ExitStack()
            inpool = inctx.enter_context(tc.tile_pool(name="inpool", bufs=1))
            xt_sb = inpool.tile([128, 3, 512], bf16)
            nc.sync.dma_start(out=xt_sb[:], in_=XT[:])
            inw_sb = inpool.tile([128, 3 * 512], bf16)
            nc.sync.dma_start(out=inw_sb[:], in_=INW[:])
            inb_sb = inpool.tile([128, 4], f32)
            nc.sync.dma_start(out=inb_sb[:], in_=INB[:])

            # ---- consts ----
            dkc_sb = consts.tile([128, 4], f32)
            nc.sync.dma_start(out=dkc_sb[:], in_=DKC[:])
            msk_sb = consts.tile([128, 4, 512], bf16)
            nc.sync.dma_start(out=msk_sb[:], in_=MSK.rearrange("k p c -> p k c"))
            onesc_sb = consts.tile([128, 1], bf16)
            nc.sync.dma_start(out=onesc_sb[:], in_=ONESC[:])
            onescr_sb = consts.tile([128, 1], f32r)
            nc.sync.dma_start(out=onescr_sb[:], in_=ONESCR[:])
            onesr_sb = consts.tile([1, 128], f32r)
            nc.sync.dma_start(out=onesr_sb[:], in_=ONESR[:])
            onw_sb = consts.tile([128, 4], f32)
            nc.sync.dma_start(out=onw_sb[:], in_=ONWC[:])
            onb_sb = consts.tile([128, 4], f32)
            nc.sync.dma_start(out=onb_sb[:], in_=ONBC[:])
            b1_sb = consts.tile([128, 16], f32)
            nc.sync.dma_start(out=b1_sb[:], in_=B1C[:])
            b2_sb = consts.tile([128, 4], f32)
            nc.sync.dma_start(out=b2_sb[:], in_=B2C[:])

            # ---- layer weight staging (one layer ahead) ----
            w_tiles = {}

            def stage(l):
                wmat = []
                for i in range(5):
                    wt = wpool.tile([128, 2048], bf16, tag="wmat")
                    nc.sync.dma_start(out=wt[:], in_=WST[l, i])
                    wmat.append(wt)
                u_sb = wsmall.tile([1, 4 * 512], bf16, tag="urow")
                nc.sync.dma_start(out=u_sb[:], in_=UST[l])
                bg_sb = wsmall.tile([128, 4], f32, tag="bgc")
                nc.sync.dma_start(out=bg_sb[:], in_=BG[l])
                bo_sb = wsmall.tile([128, 4], f32, tag="boc")
                nc.sync.dma_start(out=bo_sb[:], in_=BO[l])
                w_tiles[l] = (wmat, u_sb, bg_sb, bo_sb)

            stage(0)

            # ---- FFN weights staged once (shared by both WM layers) ----
            w1_sb = consts.tile([128, 4, 2048], bf16)
            nc.sync.dma_start(out=w1_sb[:], in_=W1T[:])
            w2_sb = consts.tile([128, 16 * 512], bf16)
            nc.sync.dma_start(out=w2_sb[:], in_=W2T[:])

            # ---- input projection: ht = (x @ in_w + in_b)^T ----
            ht = []
            htb = []
            for j in range(PT):
                p = psum.tile([128, 512], f32, tag="big")
                for c in range(3):
                    nc.tensor.matmul(
                        p[:], inw_sb[:, c * 512 + 128 * j : c * 512 + 128 * (j + 1)],
                        xt_sb[:, c, :], start=(c == 0), stop=(c == 2))
                hj = hpool.tile([128, 512], f32r, tag=f"ht{j}")
                nc.scalar.activation(hj[:], p[:], AF.Identity, bias=inb_sb[:, j : j + 1])
                ht.append(hj)
                hb = hbpool.tile([128, 512], bf16, tag=f"htb{j}")
                nc.vector.tensor_copy(out=hb[:], in_=hj[:])
                htb.append(hb)
            inctx.close()

            def neg_mean(sums_ps, want_b=False):
                """sums psum row -> (negmu f32r, negmub bf16|None) [1,512].

                Both on ACT (Copy with scale) so the DVE queue isn't on the
                critical path from stats to rank-1 corrections / planes."""
                negmu = spool.tile([1, 512], f32r, tag="tiny")
                nc.scalar.activation(negmu[:], sums_ps[:], AF.Copy,
                                     scale=-1.0 / D)
                negmub = None
                if want_b:
                    negmub = spool.tile([1, 512], bf16, tag="tinyb")
                    nc.scalar.activation(negmub[:], sums_ps[:], AF.Copy,
                                         scale=-1.0 / D)
                return negmu, negmub

            def rstd_from(ssq_ps, negmu):
                """ssq psum row (D*eps pre-added) + negmu -> rsqrt(var+eps).

                negmu^2 runs on ACT (Square) in parallel with the DVE
                backlog; the chain is then 7 serial DVE ops."""
                t = spool.tile([1, 512], f32, tag="tiny")
                nc.vector.tensor_mul(t[:], negmu[:], negmu[:])
                te = spool.tile([1, 512], f32, tag="tiny")
                nc.vector.tensor_scalar(te[:], t[:], 1.0, -EPS, ALU.mult, ALU.add)
                w = spool.tile([1, 512], f32, tag="tiny")
                nc.vector.scalar_tensor_tensor(w[:], ssq_ps[:], 1.0 / D, te[:],
                                               ALU.mult, ALU.subtract)
                nt = spool.tile([1, 512], i32, tag="tiny")
                nc.vector.tensor_scalar(nt[:], w[:].bitcast(i32), 1, -1,
                                        ALU.logical_shift_right, ALU.bitwise_xor)
                y0 = spool.tile([1, 512], i32, tag="tiny")
                nc.vector.tensor_scalar(y0[:], nt[:], MAGIC + 1, None, ALU.add)
                r = y0.bitcast(f32)
                for it in range(1):
                    a = spool.tile([1, 512], f32, tag="tiny")
                    nc.vector.tensor_mul(a[:], r[:], r[:])
                    b = spool.tile([1, 512], f32, tag="tiny")
                    nc.vector.tensor_mul(b[:], a[:], w[:])
                    wn = spool.tile([1, 512], f32, tag="tiny")
                    nc.vector.tensor_scalar(wn[:], b[:], -0.5, 1.5, ALU.mult, ALU.add)
                    rn = spool.tile([1, 512], f32r, tag="tiny")
                    nc.vector.tensor_mul(rn[:], r[:], wn[:])
                    r = rn
                return r

            def bcast_plane(row_f32r):
                """[1,512] f32r row -> [128,512] f32r SBUF plane (PE + ACT)."""
                p = bcps.tile([128, 512], f32, tag="bc")
                nc.tensor.matmul(p[:], onesr_sb[:], row_f32r[:], start=True, stop=True)
                sb = plpool.tile([128, 512], f32r, tag="plane")
                nc.scalar.copy(sb[:], p[:])
                return sb

            def retention(lidx):
                wmat, u_sb, bg_sb, bo_sb = w_tiles.pop(lidx)
                need_htb = lidx + 1 < NL

                # sums stats first: negmu unblocks the rank-1 corrections
                sums = bcps.tile([1, 512], f32, tag="bc")
                for j in range(PT):
                    nc.tensor.matmul(sums[:], onesc_sb[:], htb[j][:],
                                     start=(j == 0), stop=(j == PT - 1))
                negmu, negmub = neg_mean(sums, want_b=True)
                sq = []
                for j in range(PT):
                    s = sqpool.tile([128, 512], bf16, tag="sq")
                    nc.vector.tensor_mul(s[:], htb[j][:], htb[j][:])
                    sq.append(s)

                def proj_psums(ip):
                    pss = []
                    for j in range(PT):
                        p = psum.tile([128, 512], f32, tag="big")
                        if ip == 2:
                            for c in range(PT):
                                nc.tensor.matmul(
                                    p[:], htb[c][:, 128 * j : 128 * (j + 1)],
                                    wmat[2][:, c * 512 : (c + 1) * 512],
                                    start=(c == 0), stop=False)
                            nc.tensor.matmul(
                                p[:], negmub[:, 128 * j : 128 * (j + 1)],
                                u_sb[:, 1024 : 1536], start=False, stop=True)
                        else:
                            for c in range(PT):
                                nc.tensor.matmul(
                                    p[:],
                                    wmat[ip][:, c * 512 + 128 * j : c * 512 + 128 * (j + 1)],
                                    htb[c][:], start=(c == 0), stop=False)
                            nc.tensor.matmul(
                                p[:], u_sb[:, ip * 512 + 128 * j : ip * 512 + 128 * (j + 1)],
                                negmub[:], start=False, stop=True)
                        pss.append(p)
                    return pss

                # Q projection, then ssq stats, then K/V/G
                qt, kt, vn, gt = [], [], [], []
                q_ps = proj_psums(0)
                ssq = bcps.tile([1, 512], f32, tag="bc")
                for j in range(PT):
                    nc.tensor.matmul(ssq[:], onesc_sb[:], sq[j][:],
                                     start=(j == 0), stop=(j == PT - 1))
                r = rstd_from(ssq, negmu)
                # ks column layout: r2 row -> [128,4] via transpose DMA
                r2 = spool.tile([1, 512], f32, tag="tiny")
                nc.vector.tensor_mul(r2[:], r[:], r[:])
                r2c = spool.tile([128, 4], f32, tag="tinyc")
                for t_i in range(4):
                    nc.sync.dma_start(
                        out=r2c[:, t_i : t_i + 1],
                        in_=r2[:, 128 * t_i : 128 * (t_i + 1)])
                ksc = spool.tile([128, 4], f32, tag="tinyc")
                nc.vector.tensor_mul(ksc[:], r2c[:], dkc_sb[:])
                for j in range(PT):
                    t = apool.tile([128, 512], bf16, tag=f"proj0_{j}")
                    nc.scalar.copy(t[:], q_ps[j][:])
                    qt.append(t)

                k_ps = proj_psums(1)
                g_b = bcast_plane(r)
                for j in range(PT):
                    t = apool.tile([128, 512], bf16, tag=f"proj1_{j}")
                    nc.scalar.copy(t[:], k_ps[j][:])
                    kt.append(t)

                v_ps = proj_psums(2)
                for j in range(PT):
                    t = apool.tile([128, 512], bf16, tag=f"proj2_{j}")
                    nc.vector.tensor_scalar(t[:], v_ps[j][:], ksc[:, j : j + 1],
                                            None, ALU.mult)
                    vn.append(t)

                g_ps = proj_psums(3)
                tgs = []
                for j in range(PT):
                    tg = odpool.tile([128, 512], f32, tag="gtmp", bufs=4)
                    nc.vector.tensor_mul(tg[:], g_ps[j][:], g_b[:])
                    tgs.append(tg)

                def emit_sigmoids():
                    for j in range(PT):
                        t = apool.tile([128, 512], bf16, tag=f"proj3_{j}")
                        nc.scalar.activation(t[:], tgs[j][:], AF.Sigmoid,
                                             bias=bg_sb[:, j : j + 1])
                        gt.append(t)

                # prefetch next layer's weights now that wmat is being read
                if lidx + 1 < NL:
                    stage(lidx + 1)

                # scores + AV per head pair
                ret_sb = []
                for j in range(PT):
                    rs = retpool.tile([128, 512], bf16, tag=f"ret{j}")
                    ret_sb.append(rs)
                sq2 = []
                sums2 = bcps.tile([1, 512], f32, tag="bc")
                ssq2 = bcps.tile([1, 512], f32, tag="bc")
                for jt in range(PT):
                    rp = rpsum.tile([128, 512], f32, tag="ret")
                    for hh in range(2):
                        h = 2 * jt + hh
                        r0 = 64 * hh
                        at_tiles = []
                        for k_t in range(CT):
                            cs = 128 * k_t
                            npr = 512 - cs
                            sc = psum.tile([128, 512], f32, tag="big")
                            nc.tensor.matmul(
                                sc[:, 0:npr],
                                kt[jt][r0 : r0 + 64, 128 * k_t : 128 * (k_t + 1)],
                                qt[jt][r0 : r0 + 64, cs : 512],
                                start=True, stop=True)
                            at = atpool.tile([128, 512], bf16, tag="at")
                            if hh == 1 and npr > 128:
                                # odd heads: DVE masks the diagonal block,
                                # ACT copies the all-ones region
                                nc.vector.tensor_mul(
                                    at[:, cs : cs + 128], sc[:, 0:128],
                                    msk_sb[:, k_t, cs : cs + 128])
                                nc.scalar.copy(at[:, cs + 128 : 512],
                                               sc[:, 128:npr])
                            else:
                                nc.vector.tensor_mul(at[:, cs : 512], sc[:, 0:npr],
                                                     msk_sb[:, k_t, cs : 512])
                            at_tiles.append(at)
                        for k_t in range(CT):
                            cs = 128 * k_t
                            nc.tensor.matmul(
                                rp[r0 : r0 + 64, cs : 512],
                                vn[k_t][:, 64 * h : 64 * (h + 1)],
                                at_tiles[k_t][:, cs : 512],
                                start=(k_t == 0), stop=(k_t == CT - 1))
                    nc.scalar.copy(ret_sb[jt][:], rp[:])
                    if jt == 0:
                        emit_sigmoids()
                    s2 = sqpool.tile([128, 512], bf16, tag="sq")
                    nc.gpsimd.tensor_mul(s2[:], ret_sb[jt][:], ret_sb[jt][:])
                    sq2.append(s2)
                    nc.tensor.matmul(sums2[:], onesc_sb[:], ret_sb[jt][:],
                                     start=(jt == 0), stop=(jt == PT - 1))

                # negmu2 + plane as soon as sums2 lands (rstd2 can lag)
                negmu2, _negmu2b = neg_mean(sums2)
                nm2_b = bcast_plane(negmu2)
                for jt in range(PT):
                    nc.tensor.matmul(ssq2[:], onesc_sb[:], sq2[jt][:],
                                     start=(jt == 0), stop=(jt == PT - 1))

                # gret = (ret - mu2) * g, pipelined with the O projection
                gret = []
                for c in range(PT):
                    tmpc = odpool.tile([128, 512], bf16, tag="odb")
                    nc.vector.tensor_add(tmpc[:], ret_sb[c][:], nm2_b[:])
                    gr = grpool.tile([128, 512], bf16, tag="gret")
                    nc.vector.tensor_mul(gr[:], tmpc[:], gt[c][:])
                    gret.append(gr)
                    p1s = []
                    if c == 0:
                        for j in range(PT):
                            p1 = psum.tile([128, 512], f32, tag="big")
                            p1s.append(p1)
                    if c == 0:
                        o_ps = p1s
                    for j in range(PT):
                        nc.tensor.matmul(
                            o_ps[j][:], wmat[4][:, c * 512 + 128 * j : c * 512 + 128 * (j + 1)],
                            gret[c][:], start=(c == 0), stop=(c == PT - 1))
                rB = rstd_from(ssq2, negmu2)
                rstd2_b = bcast_plane(rB)
                if lidx + 1 < NL:
                    # HAM heartbeat: a tiny matmul gated on the plane copy
                    # lands mid-boundary so no PE-idle window reaches the
                    # ~3.4us MID threshold that re-throttles the clock
                    dm = rpsum.tile([128, 512], f32, tag="ret")
                    nc.tensor.matmul(dm[0:1, :], onescr_sb[0:1, 0:1],
                                     rstd2_b[0:1, :], start=True, stop=True)
                for j in range(PT):
                    a = odpool.tile([128, 512], f32, tag="oda")
                    nc.vector.tensor_mul(a[:], o_ps[j][:], rstd2_b[:])
                    if lidx == NL - 1:
                        hn = hpool.tile([128, 512], bf16, tag=f"ht{j}")
                        nc.vector.scalar_tensor_tensor(
                            hn[:], a[:], bo_sb[:, j : j + 1], ht[j][:],
                            ALU.add, ALU.add)
                        ht[j] = hn
                    else:
                        # DVE emits the bf16 working copy directly (next
                        # layer's matmuls need only this); the f32r residual
                        # is rebuilt on the idle Pool engine off-path
                        hb = hbpool.tile([128, 512], bf16, tag=f"htb{j}")
                        nc.vector.scalar_tensor_tensor(
                            hb[:], a[:], bo_sb[:, j : j + 1], ht[j][:],
                            ALU.add, ALU.add)
                        hn = hpool.tile([128, 512], f32r, tag=f"ht{j}")
                        nc.vector.scalar_tensor_tensor(
                            hn[:], a[:], bo_sb[:, j : j + 1], ht[j][:],
                            ALU.add, ALU.add)
                        htb[j] = hb
                        ht[j] = hn

            def ffn():
                f2ps = []
                for _j in range(PT):
                    f2p = psum.tile([128, 512], f32, tag="big")
                    f2ps.append(f2p)
                for t in range(16):
                    p = psum.tile([128, 512], f32, tag="big")
                    for c in range(PT):
                        nc.tensor.matmul(
                            p[:], w1_sb[:, c, 128 * t : 128 * (t + 1)], htb[c][:],
                            start=(c == 0), stop=(c == PT - 1))
                    f1 = f1pool.tile([128, 512], bf16, tag="f1")
                    nc.scalar.activation(f1[:], p[:], AF.Gelu, bias=b1_sb[:, t : t + 1])
                    for j in range(PT):
                        nc.tensor.matmul(
                            f2ps[j][:], w2_sb[:, t * 512 + 128 * j : t * 512 + 128 * (j + 1)],
                            f1[:], start=(t == 0), stop=(t == 15))
                for j in range(PT):
                    hb = hbpool.tile([128, 512], bf16, tag=f"htb{j}")
                    nc.vector.scalar_tensor_tensor(hb[:], f2ps[j][:],
                                                   b2_sb[:, j : j + 1],
                                                   ht[j][:], ALU.add, ALU.add)
                    htb[j] = hb
                for j in range(PT):
                    hn = hpool.tile([128, 512], f32r, tag=f"ht{j}")
                    nc.vector.scalar_tensor_tensor(hn[:], f2ps[j][:],
                                                   b2_sb[:, j : j + 1],
                                                   ht[j][:], ALU.add, ALU.add)
                    ht[j] = hn

            # world model layers
            for l in range(N_WM):
                retention(l)
                ffn()

            # final LN of world model
            sumsf = bcps.tile([1, 512], f32, tag="bc")
            for j in range(PT):
                nc.tensor.matmul(sumsf[:], onesc_sb[:], htb[j][:],
                                 start=(j == 0), stop=(j == PT - 1))
            negmuf, _negmufb = neg_mean(sumsf)
            sqf = []
            for j in range(PT):
                s = sqpool.tile([128, 512], bf16, tag="sq")
                nc.scalar.activation(s[:], htb[j][:], AF.Square)
                sqf.append(s)
            ssqf = bcps.tile([1, 512], f32, tag="bc")
            for j in range(PT):
                nc.tensor.matmul(ssqf[:], onesc_sb[:], sqf[j][:],
                                 start=(j == 0), stop=(j == PT - 1))
            rf = rstd_from(ssqf, negmuf)
            nmr = spool.tile([1, 512], f32r, tag="tiny")
            nc.vector.tensor_mul(nmr[:], negmuf[:], rf[:])
            rf_b = bcast_plane(rf)
            nmr_b = bcast_plane(nmr)
            # onw == 1 and onb == 0 (asserted host-side), so hn = ht*rf + nmr
            for j in range(PT):
                t1 = odpool.tile([128, 512], f32, tag="oda")
                nc.vector.tensor_mul(t1[:], ht[j][:], rf_b[:])
                hb = hbpool.tile([128, 512], bf16, tag=f"htb{j}")
                nc.vector.tensor_add(hb[:], t1[:], nmr_b[:])
                htb[j] = hb
                hn = hpool.tile([128, 512], f32r, tag=f"ht{j}")
                nc.vector.tensor_add(hn[:], t1[:], nmr_b[:])
                ht[j] = hn

            # retention core layers
            for l in range(N_WM, NL):
                retention(l)

            for j in range(PT):
                nc.sync.dma_start(out=HOUT[j], in_=ht[j][:])

    nc.compile()
    return nc


def _host_prep(inputs):
    """Fold weights host-side; returns the shared in_map dict (no xt)."""
    import ml_dtypes
    BF = ml_dtypes.bfloat16
    g = {k: np.asarray(v, dtype=np.float32) for k, v in inputs.items()}

    def layer_params(l):
        if l < N_WM:
            pre = "wm_"
            i = l
        else:
            pre = "co_"
            i = l - N_WM
        return {n: g[pre + n][i] for n in
                ("wq", "bq", "wk", "bk", "wv", "bv", "wg", "bg", "wo", "bo",
                 "lnw", "lnb", "prew", "preb")}

    wst = np.zeros((NL, 5, 128, 2048), BF)
    ust = np.zeros((NL, 1, 4 * 512), BF)
    bgc = np.zeros((NL, 128, 4), np.float32)
    boc = np.zeros((NL, 128, 4), np.float32)
    for l in range(NL):
        p = layer_params(l)
        wq = (p["prew"][:, None] * p["wq"]).astype(BF)
        wk = (p["prew"][:, None] * p["wk"]).astype(BF)
        wv = (p["prew"][:, None] * p["wv"]).astype(BF)
        wg = (p["prew"][:, None] * p["wg"]).astype(BF)
        wo = (p["lnw"][:, None] * p["wo"]).astype(BF)
        # biases bq~ = bq + preb @ wq must be zero for this folded fast path
        for nm, w in (("bq", p["wq"]), ("bk", p["wk"]), ("bv", p["wv"])):
            bb = p[nm] + p["preb"] @ w
            assert np.abs(bb).max() == 0.0, f"nonzero {nm} not supported"
        assert np.abs(p["lnb"]).max() == 0.0, "nonzero lnb not supported"
        bgf = p["bg"] + p["preb"] @ p["wg"]
        wst[l, 0] = _lhsT_layout(wq)
        wst[l, 1] = _lhsT_layout(wk)
        wst[l, 2] = _lhsT_layout(wv)
        wst[l, 3] = _lhsT_layout(wg)
        wst[l, 4] = _lhsT_layout(wo)
        # column sums of the bf16-rounded weights (rank-1 mean correction)
        ust[l, 0, 0:512] = wq.astype(np.float64).sum(0).astype(BF)
        ust[l, 0, 512:1024] = wk.astype(np.float64).sum(0).astype(BF)
        ust[l, 0, 1024:1536] = wv.astype(np.float64).sum(0).astype(BF)
        ust[l, 0, 1536:2048] = wg.astype(np.float64).sum(0).astype(BF)
        bgc[l] = bgf.reshape(4, 128).T
        boc[l] = p["bo"].reshape(4, 128).T

    assert np.all(g["wm_onw"] == 1.0) and np.all(g["wm_onb"] == 0.0), \
        "non-identity output norm affine not supported"
    inw = _lhsT_layout(g["in_w"].astype(BF))
    inb = g["in_b"].reshape(4, 128).T.copy()
    w1t = _lhsT_layout(g["ffn_w1"].astype(BF)).reshape(128, 4, 2048)
    w2t = _lhsT_layout(g["ffn_w2"].astype(BF))  # [128, 16*512]
    b1c = g["ffn_b1"].reshape(16, 128).T.copy()
    b2c = g["ffn_b2"].reshape(4, 128).T.copy()
    onwc = g["wm_onw"].reshape(4, 128).T.copy()
    onbc = g["wm_onb"].reshape(4, 128).T.copy()

    q = np.arange(S, dtype=np.float64)
    dkc = (DECAY ** (-q)).astype(np.float32).reshape(4, 128).T.copy()
    msk = np.zeros((4, 128, 512), BF)
    for k_t in range(4):
        msk[k_t, :, 128 * (k_t + 1):] = 1.0
        msk[k_t, :, 128 * k_t : 128 * (k_t + 1)] = np.triu(
            np.ones((128, 128), np.float32)).astype(BF)

    return {
        "inw": inw, "inb": inb, "wst": wst, "ust": ust, "bg": bgc, "bo": boc,
        "w1t": np.ascontiguousarray(w1t), "w2t": w2t, "b1c": b1c, "b2c": b2c,
        "onwc": onwc, "onbc": onbc, "dkc": dkc, "msk": msk,
        "onesc": np.ones((128, 1), BF),
        "onescr": np.ones((128, 1), np.float32),
        "epsr": np.full((1, 512), D * EPS, np.float32),
        "onesr": np.ones((1, 128), np.float32),
    }


def _in_maps(inputs):
    import ml_dtypes
    BF = ml_dtypes.bfloat16
    shared = _host_prep(inputs)
    x = np.asarray(inputs["x"], dtype=np.float32)
    in_maps = []
    for b in range(B):
        xt = np.ascontiguousarray(
            x[b].T.reshape(3, 128, 512).transpose(1, 0, 2)).astype(BF)
        m = dict(shared)
        m["xt"] = xt
        in_maps.append(m)
    return in_maps


def kernel(**inputs):
    from concourse.bass_utils import run_bass_kernel_spmd

    if "nc" not in _CACHE:
        _CACHE["nc"] = _build_program()
    nc = _CACHE["nc"]

    res = run_bass_kernel_spmd(nc, _in_maps(inputs), list(range(B)))
    out = np.empty((B, S, D), np.float32)
    for b in range(B):
        hout = res.results[b]["hout"]  # [4,128,512] = ht tiles (transposed h)
        out[b] = hout.reshape(512, 512).T.astype(np.float32)
    return out


# revision 63
# speedup vs baseline: 1.1921x; 1.1921x over previous
"""HaciCognitiveNet Trainium2 kernel (bf16 edition).

Data-parallel over batch: B=8 -> one batch element per NeuronCore.
Activations live TRANSPOSED on-chip ([D, S], D on partitions).

Matmul inputs are bf16 (fp32 PSUM accumulation): fp32 moving operands
stream at 2 cycles/column on the PE, bf16 at 1 -- halving matmul time.
The fp32 residual stream is kept in SBUF; bf16 copies (htb) feed the PE.

LayerNorm over D (partition dim) via ones-column matmuls + Newton rsqrt
on DVE. Mean correction folded into projections as rank-1 K=1 matmuls;
negmu is computed from the sums matmuls alone so projections launch
before the rsqrt chain finishes.

The decay mask 0.99^(q-k) is separable; the per-query factor
rstd(q)*0.99^q/sqrt(dh) is DROPPED entirely (the inner LayerNorm is
invariant to a positive per-token scale of ret). The per-key scale
ks(k) = rstd(k)^2 * 0.99^-k is applied on the V drain as a per-partition
ACT scale (V tiles carry keys on partitions); ks reaches column layout
via a tiny SBUF->SBUF transpose DMA. Q and K drains are plain copies.

Engine budget per retention layer (approx): PE 33us, DVE ~24us (at-mask
drains dominate), ACT ~16us. Layer-boundary serial chains are kept under
the ~3.4us HAM idle window so the PE clock stays at 2.4GHz.
"""

import numpy as np

B, S, DIN, D, H, FF = 8, 512, 384, 512, 8, 2048
DH = D // H
N_WM, N_CORE = 2, 4
NL = N_WM + N_CORE
DECAY = 0.99
EPS = 1e-5
MAGIC = 0x5F3759DF
PT = D // 128   # 4 partition tiles of the model dim
CT = S // 128   # 4 tiles of the sequence dim

_CACHE = {}


def _lhsT_layout(w):
    """[K, M] weight -> SBUF lhsT tile layout [128, (K//128)*M]."""
    k, m = w.shape
    c = k // 128
    return np.ascontiguousarray(
        w.reshape(c, 128, m).transpose(1, 0, 2).reshape(128, c * m)
    )


def _build_program():
    import concourse.bass as bass
    import concourse.tile as tile
    from concourse import mybir, bacc
    from contextlib import ExitStack

    f32 = mybir.dt.float32
    f32r = mybir.dt.float32r
    bf16 = mybir.dt.bfloat16
    i32 = mybir.dt.int32
    AF = mybir.ActivationFunctionType
    ALU = mybir.AluOpType

    nc = bacc.Bacc("TRN2", target_bir_lowering=False, debug=False)

    XT = nc.dram_tensor("xt", [128, 3, 512], bf16, kind="ExternalInput").ap()
    INW = nc.dram_tensor("inw", [128, 3 * 512], bf16, kind="ExternalInput").ap()
    INB = nc.dram_tensor("inb", [128, 4], f32, kind="ExternalInput").ap()
    WST = nc.dram_tensor("wst", [NL, 5, 128, 2048], bf16, kind="ExternalInput").ap()
    UST = nc.dram_tensor("ust", [NL, 1, 4 * 512], bf16, kind="ExternalInput").ap()
    BG = nc.dram_tensor("bg", [NL, 128, 4], f32, kind="ExternalInput").ap()
    BO = nc.dram_tensor("bo", [NL, 128, 4], f32, kind="ExternalInput").ap()
    W1T = nc.dram_tensor("w1t", [128, 4, 2048], bf16, kind="ExternalInput").ap()
    W2T = nc.dram_tensor("w2t", [128, 16 * 512], bf16, kind="ExternalInput").ap()
    B1C = nc.dram_tensor("b1c", [128, 16], f32, kind="ExternalInput").ap()
    B2C = nc.dram_tensor("b2c", [128, 4], f32, kind="ExternalInput").ap()
    ONWC = nc.dram_tensor("onwc", [128, 4], f32, kind="ExternalInput").ap()
    ONBC = nc.dram_tensor("onbc", [128, 4], f32, kind="ExternalInput").ap()
    DKC = nc.dram_tensor("dkc", [128, 4], f32, kind="ExternalInput").ap()
    MSK = nc.dram_tensor("msk", [4, 128, 512], bf16, kind="ExternalInput").ap()
    ONESC = nc.dram_tensor("onesc", [128, 1], bf16, kind="ExternalInput").ap()
    ONESCR = nc.dram_tensor("onescr", [128, 1], f32r, kind="ExternalInput").ap()
    EPSR = nc.dram_tensor("epsr", [1, 512], f32r, kind="ExternalInput").ap()
    ONESR = nc.dram_tensor("onesr", [1, 128], f32r, kind="ExternalInput").ap()
    HOUT = nc.dram_tensor("hout", [4, 128, 512], bf16, kind="ExternalOutput").ap()

    with tile.TileContext(nc) as tc:
        with ExitStack() as ctx:
            consts = ctx.enter_context(tc.tile_pool(name="consts", bufs=1))
            wpool = ctx.enter_context(tc.tile_pool(name="wpool", bufs=10))
            wsmall = ctx.enter_context(tc.tile_pool(name="wsmall", bufs=2))
            hpool = ctx.enter_context(tc.tile_pool(name="hpool", bufs=2))
            hbpool = ctx.enter_context(tc.tile_pool(name="hbpool", bufs=2))
            apool = ctx.enter_context(tc.tile_pool(name="apool", bufs=1))
            atpool = ctx.enter_context(tc.tile_pool(name="atpool", bufs=8))
            spool = ctx.enter_context(tc.tile_pool(name="spool", bufs=6))
            sqpool = ctx.enter_context(tc.tile_pool(name="sqpool", bufs=4))
            grpool = ctx.enter_context(tc.tile_pool(name="grpool", bufs=4))
            retpool = ctx.enter_context(tc.tile_pool(name="retpool", bufs=1))
            plpool = ctx.enter_context(tc.tile_pool(name="plpool", bufs=4))
            odpool = ctx.enter_context(tc.tile_pool(name="odpool", bufs=2))
            f1pool = ctx.enter_context(tc.tile_pool(name="f1pool", bufs=3))
            psum = ctx.enter_context(tc.tile_pool(name="psum", bufs=5, space="PSUM"))
            bcps = ctx.enter_context(tc.tile_pool(name="bcps", bufs=2, space="PSUM"))
            rpsum = ctx.enter_context(tc.tile_pool(name="rpsum", bufs=1, space="PSUM"))

            # ---- input DMAs first so the PE can start ASAP ----
            inctx = ExitStack()
            inpool = inctx.enter_context(tc.tile_pool(name="inpool", bufs=1))
            xt_sb = inpool.tile([128, 3, 512], bf16)
            nc.sync.dma_start(out=xt_sb[:], in_=XT[:])
            inw_sb = inpool.tile([128, 3 * 512], bf16)
            nc.sync.dma_start(out=inw_sb[:], in_=INW[:])
            inb_sb = inpool.tile([128, 4], f32)
            nc.sync.dma_start(out=inb_sb[:], in_=INB[:])

            # ---- consts ----
            dkc_sb = consts.tile([128, 4], f32)
            nc.sync.dma_start(out=dkc_sb[:], in_=DKC[:])
            msk_sb = consts.tile([128, 4, 512], bf16)
            nc.sync.dma_start(out=msk_sb[:], in_=MSK.rearrange("k p c -> p k c"))
            onesc_sb = consts.tile([128, 1], bf16)
            nc.sync.dma_start(out=onesc_sb[:], in_=ONESC[:])
            onescr_sb = consts.tile([128, 1], f32r)
            nc.sync.dma_start(out=onescr_sb[:], in_=ONESCR[:])
            onesr_sb = consts.tile([1, 128], f32r)
            nc.sync.dma_start(out=onesr_sb[:], in_=ONESR[:])
            onw_sb = consts.tile([128, 4], f32)
            nc.sync.dma_start(out=onw_sb[:], in_=ONWC[:])
            onb_sb = consts.tile([128, 4], f32)
            nc.sync.dma_start(out=onb_sb[:], in_=ONBC[:])
            b1_sb = consts.tile([128, 16], f32)
            nc.sync.dma_start(out=b1_sb[:], in_=B1C[:])
            b2_sb = consts.tile([128, 4], f32)
            nc.sync.dma_start(out=b2_sb[:], in_=B2C[:])

            # ---- layer weight staging (one layer ahead) ----
            w_tiles = {}

            def stage(l):
                wmat = []
                for i in range(5):
                    wt = wpool.tile([128, 2048], bf16, tag="wmat")
                    # chunked so the first LDWEIGHTS waits only on its slice
                    for cc in range(4):
                        nc.sync.dma_start(
                            out=wt[:, 512 * cc : 512 * (cc + 1)],
                            in_=WST[l, i, :, 512 * cc : 512 * (cc + 1)])
                    wmat.append(wt)
                u_sb = wsmall.tile([1, 4 * 512], bf16, tag="urow")
                nc.sync.dma_start(out=u_sb[:], in_=UST[l])
                bg_sb = wsmall.tile([128, 4], f32, tag="bgc")
                nc.sync.dma_start(out=bg_sb[:], in_=BG[l])
                bo_sb = wsmall.tile([128, 4], f32, tag="boc")
                nc.sync.dma_start(out=bo_sb[:], in_=BO[l])
                w_tiles[l] = (wmat, u_sb, bg_sb, bo_sb)

            stage(0)

            # ---- FFN weights staged once (shared by both WM layers) ----
            w1_sb = consts.tile([128, 4, 2048], bf16)
            nc.sync.dma_start(out=w1_sb[:], in_=W1T[:])
            w2_sb = consts.tile([128, 16 * 512], bf16)
            nc.sync.dma_start(out=w2_sb[:], in_=W2T[:])

            # ---- input projection: ht = (x @ in_w + in_b)^T ----
            ht = []
            htb = []
            for j in range(PT):
                p = psum.tile([128, 512], f32, tag="big")
                for c in range(3):
                    nc.tensor.matmul(
                        p[:], inw_sb[:, c * 512 + 128 * j : c * 512 + 128 * (j + 1)],
                        xt_sb[:, c, :], start=(c == 0), stop=(c == 2))
                hj = hpool.tile([128, 512], f32r, tag=f"ht{j}")
                nc.scalar.activation(hj[:], p[:], AF.Identity, bias=inb_sb[:, j : j + 1])
                ht.append(hj)
                hb = hbpool.tile([128, 512], bf16, tag=f"htb{j}")
                nc.vector.tensor_copy(out=hb[:], in_=hj[:])
                htb.append(hb)
            inctx.close()

            def neg_mean(sums_ps, want_b=False):
                """sums psum row -> (negmu f32r, negmub bf16|None) [1,512].

                Both on ACT (Copy with scale) so the DVE queue isn't on the
                critical path from stats to rank-1 corrections / planes."""
                negmu = spool.tile([1, 512], f32r, tag="tiny")
                nc.scalar.activation(negmu[:], sums_ps[:], AF.Copy,
                                     scale=-1.0 / D)
                negmub = None
                if want_b:
                    negmub = spool.tile([1, 512], bf16, tag="tinyb")
                    nc.scalar.activation(negmub[:], sums_ps[:], AF.Copy,
                                         scale=-1.0 / D)
                return negmu, negmub

            def rstd_from(ssq_ps, negmu):
                """ssq psum row (D*eps pre-added) + negmu -> rsqrt(var+eps).

                negmu^2 runs on ACT (Square) in parallel with the DVE
                backlog; the chain is then 7 serial DVE ops."""
                t = spool.tile([1, 512], f32, tag="tiny")
                nc.vector.tensor_mul(t[:], negmu[:], negmu[:])
                te = spool.tile([1, 512], f32, tag="tiny")
                nc.vector.tensor_scalar(te[:], t[:], 1.0, -EPS, ALU.mult, ALU.add)
                w = spool.tile([1, 512], f32, tag="tiny")
                nc.vector.scalar_tensor_tensor(w[:], ssq_ps[:], 1.0 / D, te[:],
                                               ALU.mult, ALU.subtract)
                nt = spool.tile([1, 512], i32, tag="tiny")
                nc.vector.tensor_scalar(nt[:], w[:].bitcast(i32), 1, -1,
                                        ALU.logical_shift_right, ALU.bitwise_xor)
                y0 = spool.tile([1, 512], i32, tag="tiny")
                nc.vector.tensor_scalar(y0[:], nt[:], MAGIC + 1, None, ALU.add)
                r = y0.bitcast(f32)
                for it in range(1):
                    a = spool.tile([1, 512], f32, tag="tiny")
                    nc.vector.tensor_mul(a[:], r[:], r[:])
                    b = spool.tile([1, 512], f32, tag="tiny")
                    nc.vector.tensor_mul(b[:], a[:], w[:])
                    wn = spool.tile([1, 512], f32, tag="tiny")
                    nc.vector.tensor_scalar(wn[:], b[:], -0.5, 1.5, ALU.mult, ALU.add)
                    rn = spool.tile([1, 512], f32r, tag="tiny")
                    nc.vector.tensor_mul(rn[:], r[:], wn[:])
                    r = rn
                return r

            def bcast_plane(row_f32r):
                """[1,512] f32r row -> [128,512] f32 SBUF plane (PE + ACT)."""
                p = bcps.tile([128, 512], f32, tag="bc")
                nc.tensor.matmul(p[:], onesr_sb[:], row_f32r[:], start=True, stop=True)
                sb = plpool.tile([128, 512], f32, tag="plane")
                nc.scalar.copy(sb[:], p[:])
                return sb

            def retention(lidx):
                wmat, u_sb, bg_sb, bo_sb = w_tiles.pop(lidx)
                need_htb = lidx + 1 < NL

                # sums stats first: negmu unblocks the rank-1 corrections
                sums = bcps.tile([1, 512], f32, tag="bc")
                for j in range(PT):
                    nc.tensor.matmul(sums[:], onesc_sb[:], htb[j][:],
                                     start=(j == 0), stop=(j == PT - 1))
                negmu, negmub = neg_mean(sums, want_b=True)
                sq = []
                for j in range(PT):
                    s = sqpool.tile([128, 512], bf16, tag="sq")
                    nc.vector.tensor_mul(s[:], htb[j][:], htb[j][:])
                    sq.append(s)

                def proj_psums(ip):
                    pss = []
                    for j in range(PT):
                        p = psum.tile([128, 512], f32, tag="big")
                        if ip == 2:
                            for c in range(PT):
                                nc.tensor.matmul(
                                    p[:], htb[c][:, 128 * j : 128 * (j + 1)],
                                    wmat[2][:, c * 512 : (c + 1) * 512],
                                    start=(c == 0), stop=False)
                            nc.tensor.matmul(
                                p[:], negmub[:, 128 * j : 128 * (j + 1)],
                                u_sb[:, 1024 : 1536], start=False, stop=True)
                        else:
                            for c in range(PT):
                                nc.tensor.matmul(
                                    p[:],
                                    wmat[ip][:, c * 512 + 128 * j : c * 512 + 128 * (j + 1)],
                                    htb[c][:], start=(c == 0), stop=False)
                            nc.tensor.matmul(
                                p[:], u_sb[:, ip * 512 + 128 * j : ip * 512 + 128 * (j + 1)],
                                negmub[:], start=False, stop=True)
                        pss.append(p)
                    return pss

                # Q projection, then ssq stats, then K/V/G
                qt, kt, vn, gt = [], [], [], []
                q_ps = proj_psums(0)
                ssq = bcps.tile([1, 512], f32, tag="bc")
                for j in range(PT):
                    nc.tensor.matmul(ssq[:], onesc_sb[:], sq[j][:],
                                     start=(j == 0), stop=(j == PT - 1))
                r = rstd_from(ssq, negmu)
                # ks column layout: r2 row -> [128,4] via transpose DMA
                r2 = spool.tile([1, 512], f32, tag="tiny")
                nc.vector.tensor_mul(r2[:], r[:], r[:])
                r2c = spool.tile([128, 4], f32, tag="tinyc")
                for t_i in range(4):
                    nc.sync.dma_start(
                        out=r2c[:, t_i : t_i + 1],
                        in_=r2[:, 128 * t_i : 128 * (t_i + 1)])
                ksc = spool.tile([128, 4], f32, tag="tinyc")
                nc.vector.tensor_mul(ksc[:], r2c[:], dkc_sb[:])
                for j in range(PT):
                    t = apool.tile([128, 512], bf16, tag=f"proj0_{j}")
                    nc.scalar.copy(t[:], q_ps[j][:])
                    qt.append(t)

                k_ps = proj_psums(1)
                g_b = bcast_plane(r)
                for j in range(PT):
                    t = apool.tile([128, 512], bf16, tag=f"proj1_{j}")
                    nc.scalar.copy(t[:], k_ps[j][:])
                    kt.append(t)

                v_ps = proj_psums(2)
                for j in range(PT):
                    t = apool.tile([128, 512], bf16, tag=f"proj2_{j}")
                    nc.vector.tensor_scalar(t[:], v_ps[j][:], ksc[:, j : j + 1],
                                            None, ALU.mult)
                    vn.append(t)

                g_ps = proj_psums(3)
                tgs = []
                for j in range(PT):
                    tg = odpool.tile([128, 512], f32, tag="gtmp", bufs=4)
                    nc.vector.tensor_mul(tg[:], g_ps[j][:], g_b[:])
                    tgs.append(tg)

                def emit_sigmoids():
                    for j in range(PT):
                        t = apool.tile([128, 512], bf16, tag=f"proj3_{j}")
                        nc.scalar.activation(t[:], tgs[j][:], AF.Sigmoid,
                                             bias=bg_sb[:, j : j + 1])
                        gt.append(t)

                # prefetch next layer's weights now that wmat is being read
                if lidx + 1 < NL:
                    stage(lidx + 1)

                # scores + AV per head pair
                ret_sb = []
                for j in range(PT):
                    rs = retpool.tile([128, 512], bf16, tag=f"ret{j}")
                    ret_sb.append(rs)
                sq2 = []
                sums2 = bcps.tile([1, 512], f32, tag="bc")
                ssq2 = bcps.tile([1, 512], f32, tag="bc")
                for jt in range(PT):
                    rp = rpsum.tile([128, 512], f32, tag="ret")
                    for hh in range(2):
                        h = 2 * jt + hh
                        r0 = 64 * hh
                        at_tiles = []
                        for k_t in range(CT):
                            cs = 128 * k_t
                            npr = 512 - cs
                            sc = psum.tile([128, 512], f32, tag="big")
                            nc.tensor.matmul(
                                sc[:, 0:npr],
                                kt[jt][r0 : r0 + 64, 128 * k_t : 128 * (k_t + 1)],
                                qt[jt][r0 : r0 + 64, cs : 512],
                                start=True, stop=True)
                            at = atpool.tile([128, 512], bf16, tag="at")
                            if hh == 1 and npr > 128:
                                # odd heads: DVE masks the diagonal block,
                                # ACT copies the all-ones region
                                nc.vector.tensor_mul(
                                    at[:, cs : cs + 128], sc[:, 0:128],
                                    msk_sb[:, k_t, cs : cs + 128])
                                nc.scalar.copy(at[:, cs + 128 : 512],
                                               sc[:, 128:npr])
                            else:
                                nc.vector.tensor_mul(at[:, cs : 512], sc[:, 0:npr],
                                                     msk_sb[:, k_t, cs : 512])
                            at_tiles.append(at)
                        for k_t in range(CT):
                            cs = 128 * k_t
                            nc.tensor.matmul(
                                rp[r0 : r0 + 64, cs : 512],
                                vn[k_t][:, 64 * h : 64 * (h + 1)],
                                at_tiles[k_t][:, cs : 512],
                                start=(k_t == 0), stop=(k_t == CT - 1))
                    nc.scalar.copy(ret_sb[jt][:], rp[:])
                    if jt == 0:
                        emit_sigmoids()
                    s2 = sqpool.tile([128, 512], bf16, tag="sq")
                    nc.gpsimd.tensor_mul(s2[:], ret_sb[jt][:], ret_sb[jt][:])
                    sq2.append(s2)
                    nc.tensor.matmul(sums2[:], onesc_sb[:], ret_sb[jt][:],
                                     start=(jt == 0), stop=(jt == PT - 1))

                # negmu2 + plane as soon as sums2 lands (rstd2 can lag)
                negmu2, _negmu2b = neg_mean(sums2)
                nm2_b = bcast_plane(negmu2)
                for jt in range(PT):
                    nc.tensor.matmul(ssq2[:], onesc_sb[:], sq2[jt][:],
                                     start=(jt == 0), stop=(jt == PT - 1))

                # gret = (ret - mu2) * g, pipelined with the O projection
                gret = []
                for c in range(PT):
                    tmpc = odpool.tile([128, 512], bf16, tag="odb")
                    nc.vector.tensor_add(tmpc[:], ret_sb[c][:], nm2_b[:])
                    gr = grpool.tile([128, 512], bf16, tag="gret")
                    nc.vector.tensor_mul(gr[:], tmpc[:], gt[c][:])
                    gret.append(gr)
                    p1s = []
                    if c == 0:
                        for j in range(PT):
                            p1 = psum.tile([128, 512], f32, tag="big")
                            p1s.append(p1)
                    if c == 0:
                        o_ps = p1s
                    for j in range(PT):
                        nc.tensor.matmul(
                            o_ps[j][:], wmat[4][:, c * 512 + 128 * j : c * 512 + 128 * (j + 1)],
                            gret[c][:], start=(c == 0), stop=(c == PT - 1))
                rB = rstd_from(ssq2, negmu2)
                rstd2_b = bcast_plane(rB)
                for j in range(PT):
                    a = odpool.tile([128, 512], f32, tag="oda")
                    nc.vector.tensor_mul(a[:], o_ps[j][:], rstd2_b[:])
                    if lidx == NL - 1:
                        hn = hpool.tile([128, 512], bf16, tag=f"ht{j}")
                        nc.vector.scalar_tensor_tensor(
                            hn[:], a[:], bo_sb[:, j : j + 1], ht[j][:],
                            ALU.add, ALU.add)
                        ht[j] = hn
                    else:
                        # DVE emits the bf16 working copy directly (next
                        # layer's matmuls need only this); the f32r residual
                        # is rebuilt on the idle Pool engine off-path
                        hb = hbpool.tile([128, 512], bf16, tag=f"htb{j}")
                        nc.vector.scalar_tensor_tensor(
                            hb[:], a[:], bo_sb[:, j : j + 1], ht[j][:],
                            ALU.add, ALU.add)
                        hn = hpool.tile([128, 512], f32r, tag=f"ht{j}")
                        nc.vector.scalar_tensor_tensor(
                            hn[:], a[:], bo_sb[:, j : j + 1], ht[j][:],
                            ALU.add, ALU.add)
                        htb[j] = hb
                        ht[j] = hn

            def ffn():
                f2ps = []
                for _j in range(PT):
                    f2p = psum.tile([128, 512], f32, tag="big")
                    f2ps.append(f2p)
                for t in range(16):
                    p = psum.tile([128, 512], f32, tag="big")
                    for c in range(PT):
                        nc.tensor.matmul(
                            p[:], w1_sb[:, c, 128 * t : 128 * (t + 1)], htb[c][:],
                            start=(c == 0), stop=(c == PT - 1))
                    f1 = f1pool.tile([128, 512], bf16, tag="f1")
                    nc.scalar.activation(f1[:], p[:], AF.Gelu, bias=b1_sb[:, t : t + 1])
                    for j in range(PT):
                        nc.tensor.matmul(
                            f2ps[j][:], w2_sb[:, t * 512 + 128 * j : t * 512 + 128 * (j + 1)],
                            f1[:], start=(t == 0), stop=(t == 15))
                for j in range(PT):
                    hb = hbpool.tile([128, 512], bf16, tag=f"htb{j}")
                    nc.vector.scalar_tensor_tensor(hb[:], f2ps[j][:],
                                                   b2_sb[:, j : j + 1],
                                                   ht[j][:], ALU.add, ALU.add)
                    htb[j] = hb
                for j in range(PT):
                    hn = hpool.tile([128, 512], f32r, tag=f"ht{j}")
                    nc.vector.scalar_tensor_tensor(hn[:], f2ps[j][:],
                                                   b2_sb[:, j : j + 1],
                                                   ht[j][:], ALU.add, ALU.add)
                    ht[j] = hn

            # world model layers
            for l in range(N_WM):
                retention(l)
                ffn()

            # final LN of world model
            sumsf = bcps.tile([1, 512], f32, tag="bc")
            for j in range(PT):
                nc.tensor.matmul(sumsf[:], onesc_sb[:], htb[j][:],
                                 start=(j == 0), stop=(j == PT - 1))
            negmuf, _negmufb = neg_mean(sumsf)
            sqf = []
            for j in range(PT):
                s = sqpool.tile([128, 512], bf16, tag="sq")
                nc.scalar.activation(s[:], htb[j][:], AF.Square)
                sqf.append(s)
            ssqf = bcps.tile([1, 512], f32, tag="bc")
            for j in range(PT):
                nc.tensor.matmul(ssqf[:], onesc_sb[:], sqf[j][:],
                                 start=(j == 0), stop=(j == PT - 1))
            rf = rstd_from(ssqf, negmuf)
            nmr = spool.tile([1, 512], f32r, tag="tiny")
            nc.vector.tensor_mul(nmr[:], negmuf[:], rf[:])
            rf_b = bcast_plane(rf)
            nmr_b = bcast_plane(nmr)
            # onw == 1 and onb == 0 (asserted host-side), so hn = ht*rf + nmr
            for j in range(PT):
                t1 = odpool.tile([128, 512], f32, tag="oda")
                nc.vector.tensor_mul(t1[:], ht[j][:], rf_b[:])
                hb = hbpool.tile([128, 512], bf16, tag=f"htb{j}")
                nc.vector.tensor_add(hb[:], t1[:], nmr_b[:])
                htb[j] = hb
                hn = hpool.tile([128, 512], f32r, tag=f"ht{j}")
                nc.vector.tensor_add(hn[:], t1[:], nmr_b[:])
                ht[j] = hn

            # retention core layers
            for l in range(N_WM, NL):
                retention(l)

            for j in range(PT):
                nc.sync.dma_start(out=HOUT[j], in_=ht[j][:])

    nc.compile()
    return nc


def _host_prep(inputs):
    """Fold weights host-side; returns the shared in_map dict (no xt)."""
    import ml_dtypes
    BF = ml_dtypes.bfloat16
    g = {k: np.asarray(v, dtype=np.float32) for k, v in inputs.items()}

    def layer_params(l):
        if l < N_WM:
            pre = "wm_"
            i = l
        else:
            pre = "co_"
            i = l - N_WM
        return {n: g[pre + n][i] for n in
                ("wq", "bq", "wk", "bk", "wv", "bv", "wg", "bg", "wo", "bo",
                 "lnw", "lnb", "prew", "preb")}

    wst = np.zeros((NL, 5, 128, 2048), BF)
    ust = np.zeros((NL, 1, 4 * 512), BF)
    bgc = np.zeros((NL, 128, 4), np.float32)
    boc = np.zeros((NL, 128, 4), np.float32)
    for l in range(NL):
        p = layer_params(l)
        wq = (p["prew"][:, None] * p["wq"]).astype(BF)
        wk = (p["prew"][:, None] * p["wk"]).astype(BF)
        wv = (p["prew"][:, None] * p["wv"]).astype(BF)
        wg = (p["prew"][:, None] * p["wg"]).astype(BF)
        wo = (p["lnw"][:, None] * p["wo"]).astype(BF)
        # biases bq~ = bq + preb @ wq must be zero for this folded fast path
        for nm, w in (("bq", p["wq"]), ("bk", p["wk"]), ("bv", p["wv"])):
            bb = p[nm] + p["preb"] @ w
            assert np.abs(bb).max() == 0.0, f"nonzero {nm} not supported"
        assert np.abs(p["lnb"]).max() == 0.0, "nonzero lnb not supported"
        bgf = p["bg"] + p["preb"] @ p["wg"]
        wst[l, 0] = _lhsT_layout(wq)
        wst[l, 1] = _lhsT_layout(wk)
        wst[l, 2] = _lhsT_layout(wv)
        wst[l, 3] = _lhsT_layout(wg)
        wst[l, 4] = _lhsT_layout(wo)
        # column sums of the bf16-rounded weights (rank-1 mean correction)
        ust[l, 0, 0:512] = wq.astype(np.float64).sum(0).astype(BF)
        ust[l, 0, 512:1024] = wk.astype(np.float64).sum(0).astype(BF)
        ust[l, 0, 1024:1536] = wv.astype(np.float64).sum(0).astype(BF)
        ust[l, 0, 1536:2048] = wg.astype(np.float64).sum(0).astype(BF)
        bgc[l] = bgf.reshape(4, 128).T
        boc[l] = p["bo"].reshape(4, 128).T

    assert np.all(g["wm_onw"] == 1.0) and np.all(g["wm_onb"] == 0.0), \
        "non-identity output norm affine not supported"
    inw = _lhsT_layout(g["in_w"].astype(BF))
    inb = g["in_b"].reshape(4, 128).T.copy()
    w1t = _lhsT_layout(g["ffn_w1"].astype(BF)).reshape(128, 4, 2048)
    w2t = _lhsT_layout(g["ffn_w2"].astype(BF))  # [128, 16*512]
    b1c = g["ffn_b1"].reshape(16, 128).T.copy()
    b2c = g["ffn_b2"].reshape(4, 128).T.copy()
    onwc = g["wm_onw"].reshape(4, 128).T.copy()
    onbc = g["wm_onb"].reshape(4, 128).T.copy()

    q = np.arange(S, dtype=np.float64)
    dkc = (DECAY ** (-q)).astype(np.float32).reshape(4, 128).T.copy()
    msk = np.zeros((4, 128, 512), BF)
    for k_t in range(4):
        msk[k_t, :, 128 * (k_t + 1):] = 1.0
        msk[k_t, :, 128 * k_t : 128 * (k_t + 1)] = np.triu(
            np.ones((128, 128), np.float32)).astype(BF)

    return {
        "inw": inw, "inb": inb, "wst": wst, "ust": ust, "bg": bgc, "bo": boc,
        "w1t": np.ascontiguousarray(w1t), "w2t": w2t, "b1c": b1c, "b2c": b2c,
        "onwc": onwc, "onbc": onbc, "dkc": dkc, "msk": msk,
        "onesc": np.ones((128, 1), BF),
        "onescr": np.ones((128, 1), np.float32),
        "epsr": np.full((1, 512), D * EPS, np.float32),
        "onesr": np.ones((1, 128), np.float32),
    }


def _in_maps(inputs):
    import ml_dtypes
    BF = ml_dtypes.bfloat16
    shared = _host_prep(inputs)
    x = np.asarray(inputs["x"], dtype=np.float32)
    in_maps = []
    for b in range(B):
        xt = np.ascontiguousarray(
            x[b].T.reshape(3, 128, 512).transpose(1, 0, 2)).astype(BF)
        m = dict(shared)
        m["xt"] = xt
        in_maps.append(m)
    return in_maps


def kernel(**inputs):
    from concourse.bass_utils import run_bass_kernel_spmd

    if "nc" not in _CACHE:
        _CACHE["nc"] = _build_program()
    nc = _CACHE["nc"]

    res = run_bass_kernel_spmd(nc, _in_maps(inputs), list(range(B)))
    out = np.empty((B, S, D), np.float32)
    for b in range(B):
        hout = res.results[b]["hout"]  # [4,128,512] = ht tiles (transposed h)
        out[b] = hout.reshape(512, 512).T.astype(np.float32)
    return out
